# revision 4
# baseline (speedup 1.0000x reference)
"""Trainium2 Bass kernel for a 3-layer GCN encoder (SPMD over 8 NeuronCores).

Math: each GCNConv is out = S*(A+I)*S*(x@W) + b with S=diag(deg^-1/2).
Aggregation (sparse scatter-add) commutes with the dense transform, so we
always aggregate at the 128-wide side:
  L0: agg(x) @ W0          (x is 128-wide)
  L1: agg(x0 @ W1)         (x0@W1 is 128-wide)
  L2: agg(x1) @ W2         (x1 is 128-wide)
dinv[src] is folded into the per-chunk assignment matrix A[e,j] =
(dst[e]==j) * dinv[src[e]]; dinv[dst] is applied on PSUM evacuation.
Scatter-add is a PE matmul psum[dst_tile] += A^T @ gathered_msg_rows.

Sharding: nodes by core (rows padded to a multiple of 128 per core); edges
partitioned by destination core; the 128-wide node table is all-gathered
between layers; gathers use dma_gather with int16 indices over banks of
<=25088 rows.
"""
import sys, os
for _p in ("/opt/trn_rl_repo",):
    if os.path.isdir(_p) and _p not in sys.path:
        sys.path.insert(0, _p)

import numpy as np
import concourse.bass as bass
import concourse.mybir as mybir
import concourse.tile as tile
import concourse.bacc as bacc
from concourse import bass_utils

mdt = mybir.dt
F32 = mdt.float32
I16 = mdt.int16
Alu = mybir.AluOpType
Act = mybir.ActivationFunctionType

NCORES = 8
D = 128          # aggregation feature width (IN_C and OUT_C//2)
DOUT = 256       # output feature width
BANK = 25088     # gather bank rows (int16 index range, multiple of 128)
GRP = 4          # dst tiles per gather group
SW = 4           # dst tiles per transform strip (512 cols)


# ----------------------------------------------------------------------------
# host-side preprocessing
# ----------------------------------------------------------------------------

def _pack16(a):
    """int idx array [slots] (slots%16==0) -> int16 [128, slots//16] in the
    dma_gather layout: idx i at [16r + i%16, i//16] for r in 0..7."""
    a = a.astype(np.int16)
    t = a.reshape(-1, 16).T            # [16, cols]
    return np.ascontiguousarray(np.tile(t, (8, 1)))


def _structure(edge_index, n):
    """Compute the static bucket structure + per-core data arrays."""
    percore = n // NCORES
    lpad = ((percore + 127) // 128) * 128
    ntiles = lpad // 128

    src = edge_index[0].astype(np.int64)
    dst = edge_index[1].astype(np.int64)
    loops = np.arange(n, dtype=np.int64)
    src_all = np.concatenate([src, loops])
    dst_all = np.concatenate([dst, loops])
    deg = np.bincount(dst_all, minlength=n).astype(np.float64)
    dinv = (1.0 / np.sqrt(deg)).astype(np.float32)

    core = dst_all // percore
    ld = dst_all - core * percore
    t_of = ld // 128
    j_of = ld - t_of * 128

    def mk(trow, table_rows):
        nbanks = (table_rows + BANK - 1) // BANK
        bank = trow // BANK
        bidx = trow - bank * BANK
        flat = (core * ntiles + t_of) * nbanks + bank
        cnt = np.bincount(flat, minlength=NCORES * ntiles * nbanks)
        cnt = cnt.reshape(NCORES, ntiles, nbanks)
        nch = -(-cnt.max(axis=0) // 128)            # [ntiles, nbanks] chunks
        # global chunk order: (group, bank, tile-in-group, k)
        ngroups = (ntiles + GRP - 1) // GRP
        chunk_base = np.zeros((ntiles, nbanks), np.int64)
        gb_chunks = np.zeros((ngroups, nbanks), np.int64)
        pos = 0
        for g in range(ngroups):
            tl = range(g * GRP, min((g + 1) * GRP, ntiles))
            for b in range(nbanks):
                for t in tl:
                    chunk_base[t, b] = pos
                    pos += nch[t, b]
                gb_chunks[g, b] = pos - (chunk_base[tl[0], b] if len(tl) else pos)
        nchunks = pos
        slots = nchunks * 128

        # per-core data arrays
        percore_data = []
        for c in range(NCORES):
            m = core == c
            s_c, t_c, j_c = src_all[m], t_of[m], j_of[m]
            bank_c, bidx_c = bank[m], bidx[m]
            # position of each edge inside its (t, bank) bucket
            key = t_c * nbanks + bank_c
            order = np.argsort(key, kind="stable")
            ks = key[order]
            pos_in = np.arange(len(ks)) - np.searchsorted(ks, ks, side="left")
            slot = np.empty(len(ks), np.int64)
            slot[:] = chunk_base[t_c[order], bank_c[order]] * 128 + pos_in
            idxv = np.zeros(slots, np.int64)
            dstv = np.full(slots, -1.0, np.float32)
            dscv = np.zeros(slots, np.float32)
            idxv[slot] = bidx_c[order]
            dstv[slot] = j_c[order].astype(np.float32)
            dscv[slot] = dinv[s_c[order]]
            percore_data.append((
                _pack16(idxv),
                np.ascontiguousarray(dstv.reshape(nchunks, 128).T),
                np.ascontiguousarray(dscv.reshape(nchunks, 128).T),
            ))
        return dict(nbanks=nbanks, table_rows=table_rows, nch=nch,
                    chunk_base=chunk_base, nchunks=nchunks,
                    ngroups=ngroups, percore=percore_data)

    s0 = mk(src_all, n)                                        # table = x
    trow12 = (src_all // percore) * lpad + (src_all % percore)
    s12 = mk(trow12, NCORES * lpad)                            # table = y

    # dinv packed per core: [128, ntiles]
    dinvd = []
    for c in range(NCORES):
        dv = np.zeros(lpad, np.float32)
        dv[:percore] = dinv[c * percore:(c + 1) * percore]
        dinvd.append(np.ascontiguousarray(dv.reshape(ntiles, 128).T))

    return dict(n=n, percore=percore, lpad=lpad, ntiles=ntiles,
                s0=s0, s12=s12, dinvd=dinvd, dinv=dinv)


# ----------------------------------------------------------------------------
# device kernel
# ----------------------------------------------------------------------------

def _build(S):
    n, lpad, ntiles = S["n"], S["lpad"], S["ntiles"]
    s0, s12 = S["s0"], S["s12"]

    nc = bacc.Bacc("TRN2", target_bir_lowering=False, debug=False,
                   num_devices=NCORES)

    x_d = nc.dram_tensor("x", [n, D], F32, kind="ExternalInput")
    w0_d = nc.dram_tensor("W0", [D, DOUT], F32, kind="ExternalInput")
    w1_d = nc.dram_tensor("W1", [DOUT, D], F32, kind="ExternalInput")
    w2_d = nc.dram_tensor("W2", [D, DOUT], F32, kind="ExternalInput")
    b0p_d = nc.dram_tensor("b0p", [128, 2], F32, kind="ExternalInput")
    b1b_d = nc.dram_tensor("b1b", [128, 128], F32, kind="ExternalInput")
    b2p_d = nc.dram_tensor("b2p", [128, 2], F32, kind="ExternalInput")
    dinvd_d = nc.dram_tensor("dinvd", [128, ntiles], F32, kind="ExternalInput")
    idx0_d = nc.dram_tensor("idx0", [128, s0["nchunks"] * 8], I16, kind="ExternalInput")
    dst0_d = nc.dram_tensor("dst0", [128, s0["nchunks"]], F32, kind="ExternalInput")
    dsc0_d = nc.dram_tensor("dsc0", [128, s0["nchunks"]], F32, kind="ExternalInput")
    idx12_d = nc.dram_tensor("idx12", [128, s12["nchunks"] * 8], I16, kind="ExternalInput")
    dst12_d = nc.dram_tensor("dst12", [128, s12["nchunks"]], F32, kind="ExternalInput")
    dsc12_d = nc.dram_tensor("dsc12", [128, s12["nchunks"]], F32, kind="ExternalInput")
    outT_d = nc.dram_tensor("outT", [DOUT, lpad], F32, kind="ExternalOutput")

    ysl1_d = nc.dram_tensor("ysl1", [lpad, D], F32, kind="Internal")
    ysl2_d = nc.dram_tensor("ysl2", [lpad, D], F32, kind="Internal")
    ytab1_d = nc.dram_tensor("ytab1", [NCORES * lpad, D], F32, kind="Internal",
                             addr_space="Shared")
    ytab2_d = nc.dram_tensor("ytab2", [NCORES * lpad, D], F32, kind="Internal",
                             addr_space="Shared")
    x0T_d = nc.dram_tensor("x0T", [DOUT, lpad], F32, kind="Internal")

    with tile.TileContext(nc) as tc:
        with tc.tile_pool(name="const", bufs=1) as const, \
             tc.tile_pool(name="gath", bufs=2) as gpool, \
             tc.tile_pool(name="idxp", bufs=3) as ipool, \
             tc.tile_pool(name="tblp", bufs=2) as tpool, \
             tc.tile_pool(name="amat", bufs=6) as apool, \
             tc.tile_pool(name="sp", bufs=4) as spool, \
             tc.tile_pool(name="strip", bufs=2) as stpool, \
             tc.tile_pool(name="psA", bufs=2, space="PSUM") as psA, \
             tc.tile_pool(name="psT", bufs=2, space="PSUM") as psT, \
             tc.tile_pool(name="psX", bufs=2, space="PSUM") as psX, \
             tc.tile_pool(name="psY", bufs=2, space="PSUM") as psY:

            # --- constants ---
            iota_t = const.tile([128, 128], F32)
            nc.gpsimd.iota(iota_t[:], pattern=[[1, 128]], base=0,
                           channel_multiplier=0,
                           allow_small_or_imprecise_dtypes=True)
            iota_p = const.tile([128, 1], F32)
            nc.gpsimd.iota(iota_p[:], pattern=[[1, 1]], base=0,
                           channel_multiplier=1,
                           allow_small_or_imprecise_dtypes=True)
            ident = const.tile([128, 128], F32)
            nc.vector.tensor_scalar(out=ident[:], in0=iota_t[:],
                                    scalar1=iota_p[:], scalar2=None,
                                    op0=Alu.is_equal)
            w0_t = const.tile([128, DOUT], F32)
            nc.sync.dma_start(w0_t[:], w0_d[:, :])
            w1a_t = const.tile([128, D], F32)
            nc.sync.dma_start(w1a_t[:], w1_d[0:128, :])
            w1b_t = const.tile([128, D], F32)
            nc.sync.dma_start(w1b_t[:], w1_d[128:256, :])
            w2_t = const.tile([128, DOUT], F32)
            nc.sync.dma_start(w2_t[:], w2_d[:, :])
            b0p_t = const.tile([128, 2], F32)
            nc.sync.dma_start(b0p_t[:], b0p_d[:, :])
            b1b_t = const.tile([128, 128], F32)
            nc.sync.dma_start(b1b_t[:], b1b_d[:, :])
            b2p_t = const.tile([128, 2], F32)
            nc.sync.dma_start(b2p_t[:], b2p_d[:, :])
            dinvd_t = const.tile([128, ntiles], F32)
            nc.sync.dma_start(dinvd_t[:], dinvd_d[:, :])

            def aggregate(s, idx_d, dst_d, dsc_d, table_of_bank, epilogue):
                """Scatter-add aggregation over this core's dst tiles.

                epilogue(t, s_tile): s_tile [128,128] = dinv_dst * segsum."""
                nbanks, nch = s["nbanks"], s["nch"]
                cb, ngroups = s["chunk_base"], s["ngroups"]
                icol = 0  # running idx column offset (16 idx per column)
                for g in range(ngroups):
                    tl = list(range(g * GRP, min((g + 1) * GRP, ntiles)))
                    gts = []
                    for b in range(nbanks):
                        nidx = int(nch[tl, b].sum()) * 128
                        if nidx == 0:
                            gts.append((None, 0))
                            continue
                        it = ipool.tile([128, nidx // 16], I16, tag="idx")
                        nc.sync.dma_start(it[:], idx_d[:, icol:icol + nidx // 16])
                        gt = gpool.tile([128, nidx], F32, tag=f"g{b}")
                        nc.gpsimd.dma_gather(
                            out_ap=gt[:].rearrange("p (c e) -> p c e", e=D),
                            in_ap=table_of_bank(b),
                            idxs_ap=it[:],
                            num_idxs=nidx,
                            num_idxs_reg=nidx,
                            elem_size=D,
                            single_packet=False,
                        )
                        gts.append((gt, int(cb[tl[0], b])))
                        icol += nidx // 16
                    kbase = int(cb[tl[0], 0])
                    kcnt = int(nch[tl, :].sum())
                    dt_ = tpool.tile([128, kcnt], F32, tag="dst")
                    nc.sync.dma_start(dt_[:], dst_d[:, kbase:kbase + kcnt])
                    sc_ = tpool.tile([128, kcnt], F32, tag="dsc")
                    nc.sync.dma_start(sc_[:], dsc_d[:, kbase:kbase + kcnt])
                    for t in tl:
                        acc = psA.tile([128, 128], F32, tag="acc")
                        tot = int(nch[t, :].sum())
                        done = 0
                        for b in range(nbanks):
                            gt, cb0 = gts[b]
                            for k in range(int(nch[t, b])):
                                col = int(cb[t, b]) - cb0 + k
                                kc = int(cb[t, b]) - kbase + k
                                a_t = apool.tile([128, 128], F32, tag="A")
                                nc.vector.tensor_scalar(
                                    out=a_t[:], in0=iota_t[:],
                                    scalar1=dt_[:, kc:kc + 1],
                                    scalar2=sc_[:, kc:kc + 1],
                                    op0=Alu.is_equal, op1=Alu.mult)
                                nc.tensor.matmul(
                                    acc[:], a_t[:],
                                    gt[:, col * D:(col + 1) * D],
                                    start=(done == 0), stop=(done == tot - 1))
                                done += 1
                        s_t = spool.tile([128, 128], F32, tag="s")
                        nc.scalar.activation(s_t[:], acc[:], Act.Copy,
                                             scale=dinvd_t[:, t:t + 1])
                        epilogue(t, s_t)

            # --- transform helpers (strip = up to SW dst tiles, feat-major) ---
            strip_state = {}

            def strip_put(t, s_t):
                """Transpose s_t into the current strip; returns (done, base, w)."""
                si = t // SW
                w = min(SW, ntiles - si * SW)
                if t % SW == 0:
                    strip_state["tile"] = stpool.tile([128, SW * 128], F32,
                                                      tag="sT", name="sT")
                pt = psT.tile([128, 128], F32, tag="pt")
                nc.tensor.transpose(pt[:], s_t[:], ident[:])
                st = strip_state["tile"]
                nc.vector.tensor_copy(st[:, (t % SW) * 128:(t % SW + 1) * 128], pt[:])
                if t % SW == w - 1:
                    return st, si * SW, w
                return None, 0, 0

            def epi0(t, s_t):
                st, tbase, w = strip_put(t, s_t)
                if st is None:
                    return
                cols = w * 128
                cb0 = tbase * 128
                xts = []
                for fo in range(2):
                    psx = psX.tile([128, SW * 128], F32, tag="px")
                    nc.tensor.matmul(psx[:, :cols],
                                     w0_t[:, fo * 128:(fo + 1) * 128],
                                     st[:, :cols], start=True, stop=True)
                    xt = stpool.tile([128, SW * 128], F32, tag=f"x0T{fo}")
                    nc.scalar.activation(xt[:, :cols], psx[:, :cols], Act.Relu,
                                         bias=b0p_t[:, fo:fo + 1])
                    nc.sync.dma_start(
                        x0T_d[fo * 128:(fo + 1) * 128, cb0:cb0 + cols],
                        xt[:, :cols])
                    xts.append(xt)
                for jj in range(w):
                    psy = psY.tile([128, 128], F32, tag="py")
                    nc.tensor.matmul(psy[:], xts[0][:, jj * 128:(jj + 1) * 128],
                                     w1a_t[:], start=True, stop=False)
                    nc.tensor.matmul(psy[:], xts[1][:, jj * 128:(jj + 1) * 128],
                                     w1b_t[:], start=False, stop=True)
                    yt = spool.tile([128, 128], F32, tag="yt")
                    nc.vector.tensor_copy(yt[:], psy[:])
                    nc.sync.dma_start(
                        ysl1_d[(tbase + jj) * 128:(tbase + jj + 1) * 128, :],
                        yt[:])

            def epi1(t, s_t):
                u = spool.tile([128, 128], F32, tag="u")
                nc.vector.tensor_add(u[:], s_t[:], b1b_t[:])
                x1 = spool.tile([128, 128], F32, tag="x1")
                nc.scalar.activation(x1[:], u[:], Act.Relu)
                nc.sync.dma_start(ysl2_d[t * 128:(t + 1) * 128, :], x1[:])

            def epi2(t, s_t):
                st, tbase, w = strip_put(t, s_t)
                if st is None:
                    return
                cols = w * 128
                cb0 = tbase * 128
                for fo in range(2):
                    psx = psX.tile([128, SW * 128], F32, tag="px")
                    nc.tensor.matmul(psx[:, :cols],
                                     w2_t[:, fo * 128:(fo + 1) * 128],
                                     st[:, :cols], start=True, stop=True)
                    xt = stpool.tile([128, SW * 128], F32, tag="xl")
                    nc.sync.dma_start(
                        xt[:, :cols],
                        x0T_d[fo * 128:(fo + 1) * 128, cb0:cb0 + cols])
                    v = stpool.tile([128, SW * 128], F32, tag="v")
                    nc.vector.tensor_add(v[:, :cols], psx[:, :cols], xt[:, :cols])
                    ot = stpool.tile([128, SW * 128], F32, tag="ot")
                    nc.scalar.activation(ot[:, :cols], v[:, :cols], Act.Relu,
                                         bias=b2p_t[:, fo:fo + 1])
                    nc.sync.dma_start(
                        outT_d[fo * 128:(fo + 1) * 128, cb0:cb0 + cols],
                        ot[:, :cols])

            # --- layer 0: aggregate x, transform by W0, start W1 ---
            def x_bank(b):
                lo = b * BANK
                hi = min((b + 1) * BANK, n)
                return x_d[lo:hi, :]

            aggregate(s0, idx0_d, dst0_d, dsc0_d, x_bank, epi0)

            nc.gpsimd.collective_compute(
                "AllGather", Alu.bypass,
                replica_groups=[list(range(NCORES))],
                ins=[ysl1_d.ap().opt()], outs=[ytab1_d.ap().opt()])

            def y1_bank(b):
                lo = b * BANK
                hi = min((b + 1) * BANK, NCORES * lpad)
                return ytab1_d[lo:hi, :]

            aggregate(s12, idx12_d, dst12_d, dsc12_d, y1_bank, epi1)

            nc.gpsimd.collective_compute(
                "AllGather", Alu.bypass,
                replica_groups=[list(range(NCORES))],
                ins=[ysl2_d.ap().opt()], outs=[ytab2_d.ap().opt()])

            def y2_bank(b):
                lo = b * BANK
                hi = min((b + 1) * BANK, NCORES * lpad)
                return ytab2_d[lo:hi, :]

            aggregate(s12, idx12_d, dst12_d, dsc12_d, y2_bank, epi2)

    nc.finalize()
    return nc


# ----------------------------------------------------------------------------
# entry point
# ----------------------------------------------------------------------------

def kernel(x, W0, b0, W1, b1, W2, b2, edge_index, _trace=False):
    x = np.ascontiguousarray(np.asarray(x, np.float32))
    n = x.shape[0]
    S = _structure(np.asarray(edge_index), n)
    nc = _build(S)

    b0 = np.asarray(b0, np.float32)
    b1 = np.asarray(b1, np.float32)
    b2 = np.asarray(b2, np.float32)
    shared = {
        "x": x,
        "W0": np.ascontiguousarray(np.asarray(W0, np.float32)),
        "W1": np.ascontiguousarray(np.asarray(W1, np.float32)),
        "W2": np.ascontiguousarray(np.asarray(W2, np.float32)),
        "b0p": np.ascontiguousarray(b0.reshape(2, 128).T),
        "b1b": np.ascontiguousarray(np.tile(b1[None, :], (128, 1))),
        "b2p": np.ascontiguousarray(b2.reshape(2, 128).T),
    }
    in_maps = []
    for c in range(NCORES):
        i0, d0, c0 = S["s0"]["percore"][c]
        i12, d12, c12 = S["s12"]["percore"][c]
        in_maps.append({**shared,
                        "dinvd": S["dinvd"][c],
                        "idx0": i0, "dst0": d0, "dsc0": c0,
                        "idx12": i12, "dst12": d12, "dsc12": c12})

    res = bass_utils.run_bass_kernel_spmd(
        nc, in_maps, core_ids=list(range(NCORES)), trace=_trace)

    percore = S["percore"]
    out = np.empty((n, DOUT), np.float32)
    for c in range(NCORES):
        outT = res.results[c]["outT"]
        out[c * percore:(c + 1) * percore] = outT[:, :percore].T
    if _trace:
        kernel.last_results = res
    return out


# revision 8
# speedup vs baseline: 1.2360x; 1.2360x over previous
"""Trainium2 Bass kernel for a 3-layer GCN encoder (SPMD over 8 NeuronCores).

Math: GCNConv is out = S*(A+I)*S*(x@W) + b with S=diag(deg^-1/2). The sparse
aggregation commutes with the dense transform, so we aggregate at the 128-wide
side of every layer:
  L0: agg(dinv*x) @ W0        L1: agg(dinv*(x0@W1))        L2: agg(dinv*x1) @ W2
Tables are pre-scaled by dinv[src] once per node; dinv[dst] is applied on PSUM
evacuation. Scatter-add = PE matmul psum[dst_tile] += A_o^T @ gathered rows,
with A_o[e,j] = 1[dst_e==j and parity_e==o] built by one batched DVE compare
per ~8 chunks (broadcast access pattern against a static iota).

Data movement: per-edge dma_gather descriptors are the hard floor (~8ns/desc
on the GpSimd Q7, independent of row size up to 2KB). To minimize descriptor
count the tables are stored as 2-node groups (1KB rows) so the int16 index
space needs only 2 banks, and nodes are assigned to (core, tile) slots by a
degree-balanced serpentine so per-bucket 128-padding is small. Edges are
partitioned by destination core; the 128-wide table is all-gathered between
layers.
"""
import sys, os
for _p in ("/opt/trn_rl_repo",):
    if os.path.isdir(_p) and _p not in sys.path:
        sys.path.insert(0, _p)

import numpy as np
import concourse.bass as bass
import concourse.mybir as mybir
import concourse.tile as tile
import concourse.bacc as bacc
from concourse import bass_utils

mdt = mybir.dt
F32 = mdt.float32
I16 = mdt.int16
Alu = mybir.AluOpType
Act = mybir.ActivationFunctionType

NCORES = 8
D = 128          # aggregation feature width
DOUT = 256       # output feature width
GRP = 3          # dst tiles per gather group
SW = 4           # dst tiles per transform strip (512 cols)
SCH = 8          # chunks per batched compare (16 compare columns)


def _pack16(a):
    """int idx array [slots] (slots%16==0) -> int16 [128, slots//16] in the
    dma_gather layout: idx i at [16r + i%16, i//16] for r in 0..7."""
    t = a.astype(np.int16).reshape(-1, 16).T
    return np.ascontiguousarray(np.tile(t, (8, 1)))


def _assign(deg, n, percore, ntiles):
    """Degree-balanced serpentine assignment of nodes to (core, tile, pos).

    Returns node_core, node_slot (slot = tile*128 + pos) and per-core lists
    of node ids per slot (-1 for pad slots)."""
    nbins = NCORES * ntiles
    last_cap = percore - (ntiles - 1) * 128
    caps = np.full(nbins, 128, np.int64)
    caps[ntiles - 1::ntiles] = last_cap
    order = np.argsort(-deg, kind="stable")
    node_core = np.empty(n, np.int64)
    node_slot = np.empty(n, np.int64)
    pos = 0
    r = 0
    while pos < n:
        active = np.nonzero(caps > r)[0]
        if r % 2 == 1:
            active = active[::-1]
        take = active[:n - pos]
        nodes = order[pos:pos + len(take)]
        node_core[nodes] = take // ntiles
        t = take % ntiles
        node_slot[nodes] = t * 128 + r
        pos += len(take)
        r += 1
    return node_core, node_slot


def _structure(edge_index, n):
    percore = n // NCORES
    lpad = ((percore + 127) // 128) * 128
    ntiles = lpad // 128

    src = edge_index[0].astype(np.int64)
    dst = edge_index[1].astype(np.int64)
    loops = np.arange(n, dtype=np.int64)
    src_all = np.concatenate([src, loops])
    dst_all = np.concatenate([dst, loops])
    deg = np.bincount(dst_all, minlength=n).astype(np.float64)
    dinv = (1.0 / np.sqrt(deg)).astype(np.float32)

    node_core, node_slot = _assign(deg, n, percore, ntiles)

    core = node_core[dst_all]
    sl = node_slot[dst_all]
    t_of = sl // 128
    j_of = sl - t_of * 128

    ngroups_t = (ntiles + GRP - 1) // GRP

    def mk(trow, gtot):
        # bank capacity: as few banks as int16 allows, sized evenly
        nbanks = int(-(-gtot // 32000))
        cap = int(-(-gtot // nbanks))
        grp = trow // 2
        par = trow - grp * 2
        bank = grp // cap
        bidx = grp - bank * cap
        flat = (core * ntiles + t_of) * nbanks + bank
        cnt = np.bincount(flat, minlength=NCORES * ntiles * nbanks)
        cnt = cnt.reshape(NCORES, ntiles, nbanks)
        nch = -(-cnt.max(axis=0) // 128)          # [ntiles, nbanks]

        # two chunk orders: gather (g, b, t, k) and compare (g, t, b, k)
        gpos = np.zeros((ntiles, nbanks), np.int64)
        cpos = np.zeros((ntiles, nbanks), np.int64)
        pg = pc = 0
        for g in range(ngroups_t):
            tl = range(g * GRP, min((g + 1) * GRP, ntiles))
            for b in range(nbanks):
                for t in tl:
                    gpos[t, b] = pg
                    pg += nch[t, b]
            for t in tl:
                for b in range(nbanks):
                    cpos[t, b] = pc
                    pc += nch[t, b]
        nchunks = pg
        slots = nchunks * 128

        percore_data = []
        for c in range(NCORES):
            m = core == c
            t_c, j_c = t_of[m], j_of[m]
            b_c, i_c, p_c = bank[m], bidx[m], par[m]
            key = t_c * nbanks + b_c
            order = np.argsort(key, kind="stable")
            ks = key[order]
            pos_in = np.arange(len(ks)) - np.searchsorted(ks, ks, "left")
            gslot = gpos[t_c[order], b_c[order]] * 128 + pos_in
            cslot = cpos[t_c[order], b_c[order]] * 128 + pos_in
            idxv = np.zeros(slots, np.int64)
            idxv[gslot] = i_c[order]
            d2 = np.full((slots, 2), -1.0, np.float32)
            d2[cslot, p_c[order]] = j_c[order].astype(np.float32)
            # dst table [128, 2*nchunks], col = chunk*2 + o, row = pos in chunk
            d2 = d2.reshape(nchunks, 128, 2).transpose(1, 0, 2)
            percore_data.append((
                _pack16(idxv),
                np.ascontiguousarray(d2.reshape(128, 2 * nchunks)),
            ))
        return dict(nbanks=nbanks, cap=cap, nch=nch, gpos=gpos, cpos=cpos,
                    nchunks=nchunks, percore=percore_data)

    s0 = mk(src_all, n // 2)                           # table rows = x rows
    trow12 = node_core[src_all] * lpad + node_slot[src_all]
    s12 = mk(trow12, NCORES * lpad // 2)

    # per-core dinv in slot order [128, ntiles]; canonical dinv [128, xtiles]
    dinvd = []
    slot_nodes = []
    for c in range(NCORES):
        nodes = np.nonzero(node_core == c)[0]
        slots_c = node_slot[nodes]
        dv = np.zeros(lpad, np.float32)
        dv[slots_c] = dinv[nodes]
        dinvd.append(np.ascontiguousarray(dv.reshape(ntiles, 128).T))
        sn = np.full(lpad, -1, np.int64)
        sn[slots_c] = nodes
        slot_nodes.append(sn)
    xtiles = -(-n // 128)
    dvc = np.zeros(xtiles * 128, np.float32)
    dvc[:n] = dinv
    dinvc = np.ascontiguousarray(dvc.reshape(xtiles, 128).T)

    return dict(n=n, percore=percore, lpad=lpad, ntiles=ntiles,
                xtiles=xtiles, s0=s0, s12=s12, dinvd=dinvd, dinvc=dinvc,
                slot_nodes=slot_nodes)


def _build(S):
    n, lpad, ntiles, xtiles = S["n"], S["lpad"], S["ntiles"], S["xtiles"]
    s0, s12 = S["s0"], S["s12"]
    ngroups_t = (ntiles + GRP - 1) // GRP

    nc = bacc.Bacc("TRN2", target_bir_lowering=False, debug=False,
                   num_devices=NCORES)

    x_d = nc.dram_tensor("x", [n, D], F32, kind="ExternalInput")
    w0_d = nc.dram_tensor("W0", [D, DOUT], F32, kind="ExternalInput")
    w1_d = nc.dram_tensor("W1", [DOUT, D], F32, kind="ExternalInput")
    w2_d = nc.dram_tensor("W2", [D, DOUT], F32, kind="ExternalInput")
    b0p_d = nc.dram_tensor("b0p", [128, 2], F32, kind="ExternalInput")
    b1b_d = nc.dram_tensor("b1b", [128, 128], F32, kind="ExternalInput")
    b2p_d = nc.dram_tensor("b2p", [128, 2], F32, kind="ExternalInput")
    dinvd_d = nc.dram_tensor("dinvd", [128, ntiles], F32, kind="ExternalInput")
    dinvc_d = nc.dram_tensor("dinvc", [128, xtiles], F32, kind="ExternalInput")
    idx0_d = nc.dram_tensor("idx0", [128, s0["nchunks"] * 8], I16, kind="ExternalInput")
    dst0_d = nc.dram_tensor("dst0", [128, s0["nchunks"] * 2], F32, kind="ExternalInput")
    idx12_d = nc.dram_tensor("idx12", [128, s12["nchunks"] * 8], I16, kind="ExternalInput")
    dst12_d = nc.dram_tensor("dst12", [128, s12["nchunks"] * 2], F32, kind="ExternalInput")
    outT_d = nc.dram_tensor("outT", [DOUT, lpad], F32, kind="ExternalOutput")

    y0_d = nc.dram_tensor("y0", [n, D], F32, kind="Internal")
    ysl1_d = nc.dram_tensor("ysl1", [lpad, D], F32, kind="Internal")
    ysl2_d = nc.dram_tensor("ysl2", [lpad, D], F32, kind="Internal")
    ytab1_d = nc.dram_tensor("ytab1", [NCORES * lpad, D], F32, kind="Internal",
                             addr_space="Shared")
    ytab2_d = nc.dram_tensor("ytab2", [NCORES * lpad, D], F32, kind="Internal",
                             addr_space="Shared")
    x0T_d = nc.dram_tensor("x0T", [DOUT, lpad], F32, kind="Internal")

    with tile.TileContext(nc) as tc:
        with tc.tile_pool(name="const", bufs=1) as const, \
             tc.tile_pool(name="gath", bufs=2) as gpool, \
             tc.tile_pool(name="idxp", bufs=3) as ipool, \
             tc.tile_pool(name="tblp", bufs=2) as tpool, \
             tc.tile_pool(name="amat", bufs=3) as apool, \
             tc.tile_pool(name="sp", bufs=4) as spool, \
             tc.tile_pool(name="strip", bufs=2) as stpool, \
             tc.tile_pool(name="pre", bufs=2) as ppool, \
             tc.tile_pool(name="psA", bufs=2, space="PSUM") as psA, \
             tc.tile_pool(name="psT", bufs=2, space="PSUM") as psT, \
             tc.tile_pool(name="psX", bufs=2, space="PSUM") as psX, \
             tc.tile_pool(name="psY", bufs=2, space="PSUM") as psY:

            # --- constants ---
            iota_w = const.tile([128, 2 * SCH * 128], F32)
            nc.gpsimd.iota(iota_w[:], pattern=[[0, 2 * SCH], [1, 128]], base=0,
                           channel_multiplier=0,
                           allow_small_or_imprecise_dtypes=True)
            iota_p = const.tile([128, 1], F32)
            nc.gpsimd.iota(iota_p[:], pattern=[[1, 1]], base=0,
                           channel_multiplier=1,
                           allow_small_or_imprecise_dtypes=True)
            ident = const.tile([128, 128], F32)
            nc.vector.tensor_scalar(out=ident[:], in0=iota_w[:, :128],
                                    scalar1=iota_p[:], scalar2=None,
                                    op0=Alu.is_equal)
            w0_t = const.tile([128, DOUT], F32)
            nc.sync.dma_start(w0_t[:], w0_d[:, :])
            w1a_t = const.tile([128, D], F32)
            nc.sync.dma_start(w1a_t[:], w1_d[0:128, :])
            w1b_t = const.tile([128, D], F32)
            nc.sync.dma_start(w1b_t[:], w1_d[128:256, :])
            w2_t = const.tile([128, DOUT], F32)
            nc.sync.dma_start(w2_t[:], w2_d[:, :])
            b0p_t = const.tile([128, 2], F32)
            nc.sync.dma_start(b0p_t[:], b0p_d[:, :])
            b1b_t = const.tile([128, 128], F32)
            nc.sync.dma_start(b1b_t[:], b1b_d[:, :])
            b2p_t = const.tile([128, 2], F32)
            nc.sync.dma_start(b2p_t[:], b2p_d[:, :])
            dinvd_t = const.tile([128, ntiles], F32)
            nc.sync.dma_start(dinvd_t[:], dinvd_d[:, :])
            dinvc_t = const.tile([128, xtiles], F32)
            nc.sync.dma_start(dinvc_t[:], dinvc_d[:, :])

            # --- L0 pre-pass: y0 = dinv * x (canonical layout) ---
            PB = 4  # canonical tiles per batch
            full_tiles = n // 128
            for i0 in range(0, full_tiles, PB):
                nb = min(PB, full_tiles - i0)
                cols = nb * 128
                xt = ppool.tile([128, PB * 128], F32, tag="px", name="px")
                src_ap = x_d[0:full_tiles * 128, :].rearrange(
                    "(a p) d -> p a d", p=128)
                nc.sync.dma_start(xt[:, :cols], src_ap[:, i0:i0 + nb, :])
                yt = ppool.tile([128, PB * 128], F32, tag="py", name="py")
                nc.vector.tensor_tensor(
                    out=yt[:, :cols].rearrange("p (a d) -> p a d", d=D),
                    in0=xt[:, :cols].rearrange("p (a d) -> p a d", d=D),
                    in1=dinvc_t[:, i0:i0 + nb].broadcast_to([128, nb, D]),
                    op=Alu.mult)
                dst_ap = y0_d[0:full_tiles * 128, :].rearrange(
                    "(a p) d -> p a d", p=128)
                nc.scalar.dma_start(dst_ap[:, i0:i0 + nb, :], yt[:, :cols])
            rem = n - full_tiles * 128
            if rem:
                xt = ppool.tile([128, 128], F32, tag="px", name="pxr")
                nc.sync.dma_start(xt[:rem, :], x_d[full_tiles * 128:n, :])
                yt = ppool.tile([128, 128], F32, tag="py", name="pyr")
                nc.vector.tensor_scalar(
                    out=yt[:rem, :], in0=xt[:rem, :],
                    scalar1=dinvc_t[:rem, full_tiles:full_tiles + 1],
                    scalar2=None, op0=Alu.mult)
                nc.scalar.dma_start(y0_d[full_tiles * 128:n, :], yt[:rem, :])

            def aggregate(s, idx_d, dst_d, table, trows, epilogue):
                """table: DRAM AP [rows, D] viewed as 2-node groups."""
                nbanks, cap, nch = s["nbanks"], s["cap"], s["nch"]
                gpos, cpos = s["gpos"], s["cpos"]
                grp_view = table.rearrange("(g two) d -> g (two d)", two=2)
                gtot = trows // 2
                icol = 0
                for g in range(ngroups_t):
                    tl = list(range(g * GRP, min((g + 1) * GRP, ntiles)))
                    gts = []
                    for b in range(nbanks):
                        nidx = int(nch[tl, b].sum()) * 128
                        if nidx == 0:
                            gts.append((None, 0))
                            continue
                        it = ipool.tile([128, nidx // 16], I16, tag="idx",
                                        name="it")
                        nc.sync.dma_start(it[:], idx_d[:, icol:icol + nidx // 16])
                        gt = gpool.tile([128, nidx * 2], F32, tag=f"g{b}",
                                        name="gt")
                        lo = b * cap
                        hi = min((b + 1) * cap, gtot)
                        nc.gpsimd.dma_gather(
                            out_ap=gt[:].rearrange("p (c e) -> p c e", e=2 * D),
                            in_ap=grp_view[lo:hi],
                            idxs_ap=it[:],
                            num_idxs=nidx, num_idxs_reg=nidx,
                            elem_size=2 * D, single_packet=False)
                        gts.append((gt, int(gpos[tl[0], b])))
                        icol += nidx // 16
                    kbase = int(cpos[tl[0], 0])
                    kcnt = int(nch[tl, :].sum())
                    dt_ = tpool.tile([128, 2 * kcnt], F32, tag="dst", name="dt")
                    nc.sync.dma_start(dt_[:],
                                      dst_d[:, 2 * kbase:2 * (kbase + kcnt)])
                    for t in tl:
                        tbase = int(cpos[t, 0]) - kbase   # chunks before tile t
                        tcnt = int(nch[t, :].sum())
                        # batched compares: A strips over the tile's chunks
                        strips = []
                        for sb in range(0, tcnt, SCH):
                            sw = min(SCH, tcnt - sb)
                            a_t = apool.tile([128, 2 * SCH * 128], F32,
                                             tag="A", name="at")
                            c0 = 2 * (tbase + sb)
                            nc.vector.tensor_tensor(
                                out=a_t[:, :sw * 256].rearrange(
                                    "p (s e) -> p s e", e=128),
                                in0=iota_w[:, :sw * 256].rearrange(
                                    "p (s e) -> p s e", e=128),
                                in1=dt_[:, c0:c0 + 2 * sw].broadcast_to(
                                    [128, 2 * sw, 128]),
                                op=Alu.is_equal)
                            strips.append(a_t)
                        acc = psA.tile([128, 128], F32, tag="acc", name="acc")
                        done = 0
                        for b in range(nbanks):
                            gt, gb0 = gts[b]
                            tb = int(cpos[t, b]) - kbase
                            for k in range(int(nch[t, b])):
                                gcol = int(gpos[t, b]) - gb0 + k
                                ccol = tb + k - tbase   # chunk idx within tile
                                a_t = strips[ccol // SCH]
                                boff = (ccol % SCH) * 256
                                for o in range(2):
                                    nc.tensor.matmul(
                                        acc[:],
                                        a_t[:, boff + o * 128:boff + (o + 1) * 128],
                                        gt[:, gcol * 256 + o * 128:
                                           gcol * 256 + (o + 1) * 128],
                                        start=(done == 0),
                                        stop=(done == 2 * tcnt - 1))
                                    done += 1
                        s_t = spool.tile([128, 128], F32, tag="s", name="st")
                        nc.scalar.activation(s_t[:], acc[:], Act.Copy,
                                             scale=dinvd_t[:, t:t + 1])
                        epilogue(t, s_t)

            # --- transform helpers ---
            strip_state = {}

            def strip_put(t, s_t):
                si = t // SW
                w = min(SW, ntiles - si * SW)
                if t % SW == 0:
                    strip_state["tile"] = stpool.tile([128, SW * 128], F32,
                                                      tag="sT", name="sT")
                pt = psT.tile([128, 128], F32, tag="pt", name="pt")
                nc.tensor.transpose(pt[:], s_t[:], ident[:])
                st = strip_state["tile"]
                nc.vector.tensor_copy(st[:, (t % SW) * 128:(t % SW + 1) * 128],
                                      pt[:])
                if t % SW == w - 1:
                    return st, si * SW, w
                return None, 0, 0

            def epi0(t, s_t):
                st, tbase, w = strip_put(t, s_t)
                if st is None:
                    return
                cols = w * 128
                cb0 = tbase * 128
                xts = []
                for fo in range(2):
                    psx = psX.tile([128, SW * 128], F32, tag="px", name="psx")
                    nc.tensor.matmul(psx[:, :cols],
                                     w0_t[:, fo * 128:(fo + 1) * 128],
                                     st[:, :cols], start=True, stop=True)
                    xt = stpool.tile([128, SW * 128], F32, tag=f"x0T{fo}",
                                     name="xt")
                    nc.scalar.activation(xt[:, :cols], psx[:, :cols], Act.Relu,
                                         bias=b0p_t[:, fo:fo + 1])
                    nc.sync.dma_start(
                        x0T_d[fo * 128:(fo + 1) * 128, cb0:cb0 + cols],
                        xt[:, :cols])
                    xts.append(xt)
                for jj in range(w):
                    psy = psY.tile([128, 128], F32, tag="py", name="psy")
                    nc.tensor.matmul(psy[:], xts[0][:, jj * 128:(jj + 1) * 128],
                                     w1a_t[:], start=True, stop=False)
                    nc.tensor.matmul(psy[:], xts[1][:, jj * 128:(jj + 1) * 128],
                                     w1b_t[:], start=False, stop=True)
                    yt = spool.tile([128, 128], F32, tag="yt", name="yt")
                    nc.scalar.activation(yt[:], psy[:], Act.Copy,
                                         scale=dinvd_t[:, tbase + jj:tbase + jj + 1])
                    nc.scalar.dma_start(
                        ysl1_d[(tbase + jj) * 128:(tbase + jj + 1) * 128, :],
                        yt[:])

            def epi1(t, s_t):
                u = spool.tile([128, 128], F32, tag="u", name="u")
                nc.vector.tensor_add(u[:], s_t[:], b1b_t[:])
                x1 = spool.tile([128, 128], F32, tag="x1", name="x1")
                nc.scalar.activation(x1[:], u[:], Act.Relu,
                                     scale=dinvd_t[:, t:t + 1])
                nc.scalar.dma_start(ysl2_d[t * 128:(t + 1) * 128, :], x1[:])

            def epi2(t, s_t):
                st, tbase, w = strip_put(t, s_t)
                if st is None:
                    return
                cols = w * 128
                cb0 = tbase * 128
                for fo in range(2):
                    psx = psX.tile([128, SW * 128], F32, tag="px", name="psx2")
                    nc.tensor.matmul(psx[:, :cols],
                                     w2_t[:, fo * 128:(fo + 1) * 128],
                                     st[:, :cols], start=True, stop=True)
                    xt = stpool.tile([128, SW * 128], F32, tag="xl", name="xl")
                    nc.sync.dma_start(
                        xt[:, :cols],
                        x0T_d[fo * 128:(fo + 1) * 128, cb0:cb0 + cols])
                    v = stpool.tile([128, SW * 128], F32, tag="v", name="v")
                    nc.vector.tensor_add(v[:, :cols], psx[:, :cols],
                                         xt[:, :cols])
                    ot = stpool.tile([128, SW * 128], F32, tag="ot", name="ot")
                    nc.scalar.activation(ot[:, :cols], v[:, :cols], Act.Relu,
                                         bias=b2p_t[:, fo:fo + 1])
                    nc.sync.dma_start(
                        outT_d[fo * 128:(fo + 1) * 128, cb0:cb0 + cols],
                        ot[:, :cols])

            aggregate(s0, idx0_d, dst0_d, y0_d[:, :], n, epi0)

            nc.gpsimd.collective_compute(
                "AllGather", Alu.bypass,
                replica_groups=[list(range(NCORES))],
                ins=[ysl1_d.ap().opt()], outs=[ytab1_d.ap().opt()])

            aggregate(s12, idx12_d, dst12_d, ytab1_d[:, :], NCORES * lpad, epi1)

            nc.gpsimd.collective_compute(
                "AllGather", Alu.bypass,
                replica_groups=[list(range(NCORES))],
                ins=[ysl2_d.ap().opt()], outs=[ytab2_d.ap().opt()])

            aggregate(s12, idx12_d, dst12_d, ytab2_d[:, :], NCORES * lpad, epi2)

    nc.finalize()
    return nc


def kernel(x, W0, b0, W1, b1, W2, b2, edge_index, _trace=False):
    x = np.ascontiguousarray(np.asarray(x, np.float32))
    n = x.shape[0]
    S = _structure(np.asarray(edge_index), n)
    nc = _build(S)

    b0 = np.asarray(b0, np.float32)
    b1 = np.asarray(b1, np.float32)
    b2 = np.asarray(b2, np.float32)
    shared = {
        "x": x,
        "W0": np.ascontiguousarray(np.asarray(W0, np.float32)),
        "W1": np.ascontiguousarray(np.asarray(W1, np.float32)),
        "W2": np.ascontiguousarray(np.asarray(W2, np.float32)),
        "b0p": np.ascontiguousarray(b0.reshape(2, 128).T),
        "b1b": np.ascontiguousarray(np.tile(b1[None, :], (128, 1))),
        "b2p": np.ascontiguousarray(b2.reshape(2, 128).T),
        "dinvc": S["dinvc"],
    }
    in_maps = []
    for c in range(NCORES):
        i0, d0 = S["s0"]["percore"][c]
        i12, d12 = S["s12"]["percore"][c]
        in_maps.append({**shared,
                        "dinvd": S["dinvd"][c],
                        "idx0": i0, "dst0": d0,
                        "idx12": i12, "dst12": d12})

    res = bass_utils.run_bass_kernel_spmd(
        nc, in_maps, core_ids=list(range(NCORES)), trace=_trace)

    out = np.empty((n, DOUT), np.float32)
    for c in range(NCORES):
        outT = res.results[c]["outT"]
        sn = S["slot_nodes"][c]
        m = sn >= 0
        out[sn[m]] = outT[:, np.nonzero(m)[0]].T
    if _trace:
        kernel.last_results = res
    return out
